# revision 11
# baseline (speedup 1.0000x reference)
"""Trainium2 Bass kernel for nn_BentPrototypeQuantizer.

The reference quantizes each 6-dim token to its nearest codebook row. The
codebook produced by ``_bent_codebook(64)`` is *all* 64 vertices of
{-1,+1}^6 in lexicographic order, so nearest-vertex quantization decomposes
per coordinate: q_d = sign(x_d) (with the reference's fp32-tie behavior
giving -1 for |x_d| below ~1e-7 — population ~0.2 elements per run, far
inside the 2e-2 rel-err budget).

Profile-window model (validated against gauge's find_useful_time_range on
this problem's NTFFs across several kernel variants):

    exec_time = (end of the LAST kernel instruction on any engine
                 - first compute instruction)
                + ~7.5 us of fixed NRT exit protocol (each engine zeroes
                  its slice of all 256 semaphores one EVENT_SEMAPHORE at
                  a time, then a barrier — runtime-injected, identical
                  for every NEFF, not reducible kernel-side).

Loads before the first compute instruction are free; store DMA *data*
drain is also free (it completes inside the exit protocol); but each
store's HWDGE descriptor generation (PDMA2D, ~5 ns per partition
descriptor + fixed) IS a kernel instruction and delays the window end.

Structure:
1. One full-bandwidth HWDGE load (3.07 MB + the ACT bias column riding
   as column 6144). Window closed during the ~10.5 us load.
2. Compute burst gated on load completion, split DVE/ACT so both finish
   together:
   - DVE (3778 cols): one tensor_scalar on a uint32 view,
     (x & 0x80000000) | 0x3F800000 -> exact +-1.0f; two ALU stages in
     one instruction at the 2x perf mode (~0.555 ns/col measured).
   - ACT (2366 cols): Sign activation, bias = -TAU from the bias column.
     NO in-kernel ACT_TABLE_LOAD: the NEFF ships only the
     exp_and_others table set (contains Sign) and the runtime preamble
     loads it every execution, so the bass-inserted ATL is redundant —
     insert_act_table_loads is suppressed. (If a runtime stopped
     preloading, the bit-exact correctness check in test.py fails loudly.)
3. Stores split by PARTITION halves, one store per HWDGE ring (Sync ring
   takes partitions 0-63, Scalar ring 64-127, each covering all 6144
   cols): the two 64-descriptor generations run in parallel, halving the
   post-compute descgen tail vs a single 128-descriptor store.

The init-time all_engine_barrier is suppressed along with const-AP
memsets: it would put instructions on PE/Pool and the walrus pre-main
protocol already syncs all engines before our first instruction.
"""

import time

import numpy as np

import concourse.bass as bass
import concourse.bacc as bacc
from concourse import mybir
from concourse.bass_utils import run_bass_kernel_spmd

B, N, D = 32, 32768, 6
N_CORES = 8
TAU = 3e-7

ELEMS = B * N * D                      # 6291456 f32 total
PER_CORE = ELEMS // N_CORES            # 786432 f32 per core
P = 128                                # SBUF partitions
TOT_F = PER_CORE // P                  # 6144 f32 per partition
BIAS_COL = TOT_F                       # bias rides the x load as col 6144

# DVE ~0.563 ns/col (2x mode) vs ACT ~0.92 ns/col (measured), balanced so
# the two store/drain chains end together: the Scalar ring's store chain
# (descgen+drain ~1.35 us) is ~230 ns heavier than the Sync ring's, so
# DVE gets the extra work and finishes that much later.
W_ACT = 2210
W_DVE = TOT_F - W_ACT                  # 3934

SIGN_MASK = 0x80000000                 # f32 sign bit
ONE_BITS = 0x3F800000                  # f32 +1.0


def _build_nc(keep_barrier: bool = False, keep_atl: bool = False):
    owner = bass.BassEitherVectorEngine
    saved_memset = owner.memset
    saved_barrier = bass.Bass.all_engine_barrier
    owner.memset = lambda self, ap, c: None
    if not keep_barrier:
        bass.Bass.all_engine_barrier = lambda self, sem_only=False: None
    try:
        nc = bacc.Bacc(
            "TRN2",
            target_bir_lowering=False,
            debug=False,
            enable_asserts=False,
            num_devices=N_CORES,
        )
    finally:
        owner.memset = saved_memset
        bass.Bass.all_engine_barrier = saved_barrier
    if not keep_atl:
        # The runtime preamble loads the NEFF's (only) ACT table set each
        # execution; skip the redundant in-window ACT_TABLE_LOAD.
        nc.insert_act_table_loads = lambda: None

    x = nc.dram_tensor("x", [P, TOT_F + 1], mybir.dt.float32, kind="ExternalInput")
    y = nc.dram_tensor("y", [P, TOT_F], mybir.dt.float32, kind="ExternalOutput")

    tin = nc.alloc_sbuf_tensor("tin", [P, TOT_F + 1], mybir.dt.float32)
    tout = nc.alloc_sbuf_tensor("tout", [P, TOT_F], mybir.dt.float32)

    lx = nc.alloc_semaphore("lx")
    cp_dve = nc.alloc_semaphore("cp_dve")
    cp_act = nc.alloc_semaphore("cp_act")
    st = nc.alloc_semaphore("st")

    # Full-shard load at line rate; nothing in the window yet.
    nc.sync.dma_start(tin.ap(), x.ap()).then_inc(lx, 16)

    # DVE: (x & sign_mask) | one_bits -> exact +-1.0f, one instruction.
    tin_u = tin.ap().bitcast(mybir.dt.uint32)
    tout_u = tout.ap().bitcast(mybir.dt.uint32)
    nc.vector.wait_ge(lx, 16)
    nc.vector.tensor_scalar(
        tout_u[:, 0:W_DVE], tin_u[:, 0:W_DVE],
        SIGN_MASK, ONE_BITS,
        mybir.AluOpType.bitwise_and, mybir.AluOpType.bitwise_or,
    ).then_inc(cp_dve, 1)

    # ACT: sign(x - TAU) on the tail columns; bias column loaded with -TAU.
    nc.scalar.wait_ge(lx, 16)
    nc.scalar.sign(
        tout.ap()[:, W_DVE:TOT_F], tin.ap()[:, W_DVE:TOT_F],
        bias=tin.ap()[:, BIAS_COL : BIAS_COL + 1],
    ).then_inc(cp_act, 1)


    # Stores: one column region per HWDGE ring, each gated only on its own
    # compute sem so descriptor generation starts the moment that engine
    # finishes (descgen is ~0.6 us roughly independent of size; the two
    # rings overlap).
    nc.sync.wait_ge(cp_dve, 1)
    nc.sync.dma_start(y.ap()[:, 0:W_DVE], tout.ap()[:, 0:W_DVE]).then_inc(st, 16)
    nc.scalar.wait_ge(cp_act, 1)
    nc.scalar.dma_start(
        y.ap()[:, W_DVE:TOT_F], tout.ap()[:, W_DVE:TOT_F]
    ).then_inc(st, 16)

    nc.compile()
    return nc


_NC_CACHE = None


def make_shards(x: np.ndarray) -> list[dict[str, np.ndarray]]:
    """Per-core inputs: contiguous 1/8 slice + the ACT bias column."""
    x = np.asarray(x, dtype=np.float32)
    shards = np.ascontiguousarray(x).reshape(N_CORES, P, TOT_F)
    full = np.empty((N_CORES, P, TOT_F + 1), dtype=np.float32)
    full[:, :, :TOT_F] = shards
    full[:, :, TOT_F] = -TAU
    return [{"x": full[c]} for c in range(N_CORES)]


def kernel(x: np.ndarray, codebook: np.ndarray | None = None) -> np.ndarray:
    global _NC_CACHE
    x = np.asarray(x, dtype=np.float32)
    assert x.shape == (B, N, D), x.shape
    in_maps = make_shards(x)
    if _NC_CACHE is None:
        _NC_CACHE = _build_nc()
    nc = _NC_CACHE
    res = None
    for attempt in range(3):
        try:
            res = run_bass_kernel_spmd(
                nc,
                in_maps,
                core_ids=list(range(N_CORES)),
            )
            break
        except Exception:
            # transient device wedge (e.g. NRT_EXEC_UNIT_UNRECOVERABLE)
            if attempt == 2:
                raise
            time.sleep(3.0)
    out = np.concatenate(
        [res.results[c]["y"].reshape(-1) for c in range(N_CORES)]
    ).reshape(B, N, D)
    return out
